# revision 33
# baseline (speedup 1.0000x reference)
"""Trainium2 Bass kernel for nn_BinaryLinear (binarized 4-layer MLP + BatchNorm).

Reference computation (fp32, jax):
    h = x.reshape(-1, 3072)
    h = relu(h @ sign(W1).T); h = BN(h, g1, b1)   # BN over full 8192 batch
    h = relu(h @ sign(W2).T); h = BN(h, g2, b2)
    h = relu(h @ sign(W3).T); h = BN(h, g3, b3)
    out = h @ sign(W4).T                          # [8192, 10]

Strategy (8 NeuronCores, data-parallel over batch):
  - Host: binarize weights to bf16 (+-1 exact), pack everything partition-
    major ([128, ktiles, free]), shard x over cores (1024 rows each).
  - Device: activations feature-major [feature_part, batch_free] in SBUF.
    Each layer is K-tiled bf16 matmuls accumulating in PSUM, feature tiles
    processed in chain-groups [0-3 k-outer], [4], [5,6], [7 half-split].
  - BatchNorm stats (sum, sumsq of relu) exchanged in TWO AllGathers per
    layer: tiles {0..3} (ready ~40% into the layer; scale/shift applied
    well before the layer ends) and tiles {4..7} (ready at layer end,
    resolving under the next layer's k=0..3 matmuls, which only need the
    first AllGather's tiles). Stats DMAs and gather readbacks ride the
    sync ring (idle after the input feed); collective triggers alone ride
    the gpsimd queue so the CC's own data movement is never blocked.
  - Warmup: ~10 matmuls on zeroed scratch warm the PE clock (HAM) while
    the first XT/W1 chunks stream in.
"""
import os
import sys

for _p in ("/opt/trn_rl_repo",):
    if os.path.isdir(_p) and _p not in sys.path:
        sys.path.insert(0, _p)

import numpy as np
import ml_dtypes

from concourse import bacc, tile, mybir
from concourse import bass_utils

NCORES = 8
B = 8192
BL = B // NCORES            # 1024 rows per core
KIN = 3072
KT_IN = KIN // 128          # 24 k-tiles for layer 1
HID = 1024
JT = HID // 128             # 8 feature tiles
CLS = 10
CLSP = 16                   # padded classes
EPS = 1e-5
BF16 = mybir.dt.bfloat16
F32 = mybir.dt.float32
ADD = mybir.AluOpType.add
SUB = mybir.AluOpType.subtract
MUL = mybir.AluOpType.mult
BYP = mybir.AluOpType.bypass
RELU = mybir.ActivationFunctionType.Relu

_CACHE = {}

G1 = [0, 1, 2, 3]
G2 = [4, 5, 6, 7]
HALVES = [(0, 512), (512, 512)]


def _build():
    nc = bacc.Bacc("TRN2", target_bir_lowering=False, debug=False, num_devices=NCORES)

    xt_d = nc.dram_tensor("xt", [128, KT_IN, BL], BF16, kind="ExternalInput")
    w1_d = nc.dram_tensor("w1t", [128, KT_IN, HID], BF16, kind="ExternalInput")
    w2_d = nc.dram_tensor("w2t", [128, JT, HID], BF16, kind="ExternalInput")
    w3_d = nc.dram_tensor("w3t", [128, JT, HID], BF16, kind="ExternalInput")
    w4_d = nc.dram_tensor("w4t", [128, JT, CLSP], BF16, kind="ExternalInput")
    bnp_d = nc.dram_tensor("bnp", [128, 6 * JT], F32, kind="ExternalInput")
    out_d = nc.dram_tensor("out", [CLSP, BL], F32, kind="ExternalOutput")

    with tile.TileContext(nc) as tc:
        with (
            tc.tile_pool(name="weights", bufs=1) as wpool,
            tc.tile_pool(name="acts", bufs=1) as apool,
            tc.tile_pool(name="scratch", bufs=2) as scrpool,
            tc.tile_pool(name="stats", bufs=2) as spool,
            tc.tile_pool(name="psum", bufs=4, space="PSUM") as pspool,
            tc.tile_pool(name="dram", bufs=2, space="DRAM") as dpool,
        ):
            XT = wpool.tile([128, KT_IN, BL], BF16, tag="XT")
            W1 = wpool.tile([128, KT_IN, HID], BF16, tag="W1")
            W2 = wpool.tile([128, JT, HID], BF16, tag="W2")
            W3 = wpool.tile([128, JT, HID], BF16, tag="W3")
            W4 = wpool.tile([128, JT, CLSP], BF16, tag="W4")
            BNP = wpool.tile([128, 6 * JT], F32, tag="BNP")
            HRAW = apool.tile([128, JT, BL], BF16, tag="HRAW")
            H = apool.tile([128, JT, BL], BF16, tag="H")
            H2 = apool.tile([128, JT, BL], BF16, tag="H2")
            WZ = wpool.tile([128, 512], BF16, tag="WZ")

            # ---- warmup ----
            nc.gpsimd.memset(WZ[:], 0)
            wps = pspool.tile([128, BL], F32, tag="ps", name="ps_warm")
            for i in range(13):
                mi = nc.tensor.matmul(
                    wps[:, 0:512], WZ[:, 0:128], WZ[:, 0:512],
                    start=True, stop=True,
                )
                if i > 0:
                    mi.ins.ldweights = False

            # ---- input feed ----
            nc.sync.dma_start(BNP[:], bnp_d[:])
            feed = [1, 1] + [2] * 11
            c = 0
            for w in feed:
                nc.sync.dma_start(XT[:, c : c + w, :], xt_d[:, c : c + w, :])
                nc.scalar.dma_start(W1[:, c : c + w, :], w1_d[:, c : c + w, :])
                c += w
            assert c == KT_IN

            def mm_pair(ps, Wk, rhs, k, start, stop):
                for idx, (s, w) in enumerate(HALVES):
                    mi = nc.tensor.matmul(
                        ps[:, s : s + w], Wk, rhs[:, k, s : s + w],
                        start=start, stop=stop,
                    )
                    if idx > 0:
                        mi.ins.ldweights = False

            def relu_tile(ps, jt, S, col):
                nc.scalar.activation(
                    HRAW[:, jt, :], ps[:], RELU, accum_out=S[:, col : col + 1]
                )

            def sq_tile(jt, S, col, s=0, w=BL):
                scr = scrpool.tile([128, w], BF16, tag="scr", name=f"scr_{jt}_{s}")
                nc.vector.scalar_tensor_tensor(
                    scr[:], HRAW[:, jt, s : s + w], 0.0, HRAW[:, jt, s : s + w],
                    BYP, MUL, accum_out=S[:, col : col + 1],
                )

            def ar_start(li, gi, S, n2):
                """stats DMA (scalar ring -- waits only on the local sq
                accumulate, so it can't block relus; this keeps the sync
                ring's waits purely monotone GAT-completion waits, so a
                spread-stretched final mesh of layer l can never delay
                layer l+1's first stats send) + AllGather trigger (gpsimd)."""
                cc_in = dpool.tile([128, n2], F32, tag="cc_in", name=f"cc_in_{li}_{gi}")
                cc_out = dpool.tile(
                    [NCORES * 128, n2], F32, tag="cc_out", name=f"cc_out_{li}_{gi}"
                )
                nc.scalar.dma_start(cc_in[:], S[:])
                nc.gpsimd.collective_compute(
                    "AllGather", BYP,
                    replica_groups=[list(range(NCORES))],
                    ins=[cc_in.opt()], outs=[cc_out.opt()],
                )
                return cc_out

            def ar_finish(li, gi, jts, cc_out):
                """readback (sync ring) + tree-reduce and scale/shift math
                (vector, sqrt on scalar); returns (A, C)."""
                n = len(jts)
                n2 = 2 * n
                GAT = spool.tile([128, NCORES, n2], F32, tag="GAT",
                                 name=f"GAT_{li}_{gi}")
                nc.sync.dma_start(
                    GAT[:], cc_out.opt().rearrange("(c p) s -> p c s", p=128)
                )
                T4 = spool.tile([128, 4, n2], F32, tag="T4", name=f"T4_{li}_{gi}")
                nc.vector.tensor_tensor(T4[:], GAT[:, 0:4, :], GAT[:, 4:8, :], ADD)
                T2 = spool.tile([128, 2, n2], F32, tag="T2", name=f"T2_{li}_{gi}")
                nc.vector.tensor_tensor(T2[:], T4[:, 0:2, :], T4[:, 2:4, :], ADD)
                RS = spool.tile([128, n2], F32, tag="RS", name=f"RS_{li}_{gi}")
                nc.vector.tensor_tensor(RS[:], T2[:, 0, :], T2[:, 1, :], ADD)
                MEAN = spool.tile([128, n], F32, tag="MEAN", name=f"MEAN_{li}_{gi}")
                nc.vector.tensor_scalar_mul(MEAN[:], RS[:, 0:n], 1.0 / B)
                VPE = spool.tile([128, n], F32, tag="VPE", name=f"VPE_{li}_{gi}")
                nc.vector.tensor_scalar(
                    VPE[:], RS[:, n : 2 * n], 1.0 / B, EPS, MUL, ADD
                )
                MSQ = spool.tile([128, n], F32, tag="MSQ", name=f"MSQ_{li}_{gi}")
                nc.vector.tensor_tensor(MSQ[:], MEAN[:], MEAN[:], MUL)
                VAR = spool.tile([128, n], F32, tag="VAR", name=f"VAR_{li}_{gi}")
                nc.vector.tensor_tensor(VAR[:], VPE[:], MSQ[:], SUB)
                RINV = spool.tile([128, n], F32, tag="RINV", name=f"RINV_{li}_{gi}")
                nc.vector.reciprocal(RINV[:], VAR[:])
                RSTD = spool.tile([128, n], F32, tag="RSTD", name=f"RSTD_{li}_{gi}")
                nc.scalar.sqrt(RSTD[:], RINV[:])
                g0 = (2 * li) * JT + jts[0]
                b0 = (2 * li + 1) * JT + jts[0]
                A = spool.tile([128, n], F32, tag="A", name=f"A_{li}_{gi}")
                nc.vector.tensor_tensor(A[:], RSTD[:], BNP[:, g0 : g0 + n], MUL)
                AM = spool.tile([128, n], F32, tag="AM", name=f"AM_{li}_{gi}")
                nc.vector.tensor_tensor(AM[:], A[:], MEAN[:], MUL)
                C = spool.tile([128, n], F32, tag="C", name=f"C_{li}_{gi}")
                nc.vector.tensor_tensor(C[:], BNP[:, b0 : b0 + n], AM[:], SUB)
                return A, C

            def apply_tile(Hdst, jt, A, C, jj):
                nc.vector.tensor_scalar(
                    Hdst[:, jt, :], HRAW[:, jt, :],
                    A[:, jj : jj + 1], C[:, jj : jj + 1], MUL, ADD,
                )

            def mlp_layer(li, kt, rhs, W, Hdst):
                """One layer: chain-groups [0-3 k-outer], [4,5], [6], [7].

                THREE stats AllGathers -- {0..3}, {4,5}, {6,7} -- so the final
                one covers only tiles 6,7 and the next layer can hide its
                latency + inter-core spread under k=0..5 (~12.6us of matmuls,
                the PSUM-bounded maximum)."""
                S1 = spool.tile([128, 8], F32, tag="S1", name=f"S1_{li}")
                S45 = spool.tile([128, 4], F32, tag="S45", name=f"S45_{li}")
                S2 = spool.tile([128, 4], F32, tag="S2", name=f"S2_{li}")
                Sh = spool.tile([128, 4], F32, tag="Sh", name=f"Sh_{li}")

                # group A: feature tiles 0..3, k-outer over 4 full-batch chains
                pss = [
                    pspool.tile([128, BL], F32, tag="ps", name=f"psA{li}_{j}")
                    for j in range(4)
                ]
                for k in range(kt):
                    for j in range(4):
                        mm_pair(pss[j], W[:, k, j * 128 : (j + 1) * 128],
                                rhs, k, k == 0, k == kt - 1)
                for j in range(4):
                    relu_tile(pss[j], j, S1, j)
                    sq_tile(j, S1, 4 + j)
                cc1 = ar_start(li, 0, S1, 8)

                # group [4,5]: k-outer over 2 chains
                ps45 = [
                    pspool.tile([128, BL], F32, tag="ps", name=f"ps45_{li}_{j}")
                    for j in range(2)
                ]
                for k in range(kt):
                    for j in range(2):
                        mm_pair(ps45[j], W[:, k, (4 + j) * 128 : (5 + j) * 128],
                                rhs, k, k == 0, k == kt - 1)
                relu_tile(ps45[0], 4, S45, 0)
                sq_tile(4, S45, 2)
                relu_tile(ps45[1], 5, S45, 1)
                sq_tile(5, S45, 3)
                ccb = ar_start(li, 1, S45, 4)

                # group [6]: single chain
                ps6 = pspool.tile([128, BL], F32, tag="ps", name=f"ps6_{li}")
                for k in range(kt):
                    mm_pair(ps6, W[:, k, 6 * 128 : 7 * 128], rhs, k,
                            k == 0, k == kt - 1)
                relu_tile(ps6, 6, S2, 0)
                sq_tile(6, S2, 2)

                # group [7]: two half-batch chains sharing one psum tile
                ps7 = pspool.tile([128, BL], F32, tag="ps", name=f"ps7_{li}")
                for k in range(kt):
                    for hi, (s, w) in enumerate(HALVES):
                        mi = nc.tensor.matmul(
                            ps7[:, s : s + w], W[:, k, 7 * 128 : 8 * 128],
                            rhs[:, k, s : s + w], start=(k == 0), stop=(k == kt - 1),
                        )
                        if hi > 0:
                            mi.ins.ldweights = False
                for hi, (s, w) in enumerate(HALVES):
                    nc.scalar.activation(
                        HRAW[:, 7, s : s + w], ps7[:, s : s + w], RELU,
                        accum_out=Sh[:, hi : hi + 1],
                    )
                for hi, (s, w) in enumerate(HALVES):
                    sq_tile(7, Sh, 2 + hi, s, w)
                nc.vector.tensor_tensor(S2[:, 1:2], Sh[:, 0:1], Sh[:, 1:2], ADD)
                nc.vector.tensor_tensor(S2[:, 3:4], Sh[:, 2:3], Sh[:, 3:4], ADD)
                ccf = ar_start(li, 9, S2, 4)

                # All AllGather consume paths (tree/math/sqrt/applies) are
                # emitted AFTER every stats op above, so a slow mesh can never
                # block the relu/sq stream or the final stats trigger on the
                # in-order scalar/vector queues.
                A1, C1 = ar_finish(li, 0, G1, cc1)
                for jj in range(4):
                    apply_tile(Hdst, jj, A1, C1, jj)
                Ab, Cb = ar_finish(li, 1, [4, 5], ccb)
                apply_tile(Hdst, 4, Ab, Cb, 0)
                apply_tile(Hdst, 5, Ab, Cb, 1)
                A2, C2 = ar_finish(li, 9, [6, 7], ccf)
                apply_tile(Hdst, 6, A2, C2, 0)
                apply_tile(Hdst, 7, A2, C2, 1)

            # ---- remaining weight feeds (ahead of stats traffic in ring order)
            nc.sync.dma_start(W2[:, 0:4, :], w2_d[:, 0:4, :])
            nc.sync.dma_start(W2[:, 4:8, :], w2_d[:, 4:8, :])
            nc.sync.dma_start(W4[:], w4_d[:])
            nc.scalar.dma_start(W3[:, 0:4, :], w3_d[:, 0:4, :])
            nc.scalar.dma_start(W3[:, 4:8, :], w3_d[:, 4:8, :])

            # ---- layers ----
            mlp_layer(0, KT_IN, XT, W1, H)
            mlp_layer(1, JT, H, W2, H2)
            mlp_layer(2, JT, H2, W3, H)

            # ---- layer 4 (no relu/BN): k 0..3 run right at L3 end, k 4..7
            # after L3's second AllGather applies ----
            pso = pspool.tile([128, BL], F32, tag="ps", name="ps_out")
            psK = pspool.tile([128, BL], F32, tag="ps", name="ps_keepwarm")
            for k in range(JT):
                if k == 6:
                    # keep-warm: the PE idles here waiting for L3's final
                    # AllGather; ~3.4us of dummy matmuls hold the HAM clock
                    # at speed so k6,7 + the output copy don't run cold.
                    for i in range(8):
                        mi = nc.tensor.matmul(
                            psK[:, 0:512], WZ[:, 0:128], WZ[:, 0:512],
                            start=True, stop=True,
                        )
                        if i > 0:
                            mi.ins.ldweights = False
                mm_pair(pso[0:CLSP, :], W4[:, k, :], H, k, k == 0, k == JT - 1)
            # per-half copy+DMA so the first half's drain overlaps the second
            # half's final matmuls
            OUTS = spool.tile([CLSP, BL], F32, tag="OUTS")
            for s, w in HALVES:
                nc.scalar.copy(OUTS[:, s : s + w], pso[0:CLSP, s : s + w])
                nc.sync.dma_start(out_d[:, s : s + w], OUTS[:, s : s + w])

    nc.compile()
    return nc


def _get_nc():
    if "nc" not in _CACHE:
        _CACHE["nc"] = _build()
    return _CACHE["nc"]


def _prep_inputs(x, W1, W2, W3, W4, g1, b1, g2, b2, g3, b3):
    x2 = np.asarray(x, dtype=np.float32).reshape(B, KIN)
    xt = np.ascontiguousarray(x2.T).astype(ml_dtypes.bfloat16)  # [3072, 8192]

    def pmajor(a):
        kt = a.shape[0] // 128
        return np.ascontiguousarray(
            a.reshape(kt, 128, a.shape[1]).transpose(1, 0, 2)
        )

    def bin_t(w, pad=None):
        wb = np.where(np.asarray(w, dtype=np.float32) >= 0, 1.0, -1.0)
        wt = np.ascontiguousarray(wb.T).astype(ml_dtypes.bfloat16)  # [in, out]
        if pad is not None and wt.shape[1] < pad:
            wt = np.concatenate(
                [wt, np.zeros((wt.shape[0], pad - wt.shape[1]), wt.dtype)], axis=1
            )
        return pmajor(wt)

    w1t = bin_t(W1)
    w2t = bin_t(W2)
    w3t = bin_t(W3)
    w4t = bin_t(W4, pad=CLSP)

    bnp = np.zeros((128, 6 * JT), dtype=np.float32)
    for l, p in enumerate([g1, b1, g2, b2, g3, b3]):
        pa = np.asarray(p, dtype=np.float32)
        for jt in range(JT):
            bnp[:, l * JT + jt] = pa[jt * 128 : (jt + 1) * 128]

    shared = {"w1t": w1t, "w2t": w2t, "w3t": w3t, "w4t": w4t, "bnp": bnp}
    in_maps = []
    for c in range(NCORES):
        m = dict(shared)
        m["xt"] = pmajor(np.ascontiguousarray(xt[:, c * BL : (c + 1) * BL]))
        in_maps.append(m)
    return in_maps


def _run(inputs, trace=False):
    nc = _get_nc()
    in_maps = _prep_inputs(**inputs)
    res = bass_utils.run_bass_kernel_spmd(
        nc, in_maps, core_ids=list(range(NCORES)), trace=trace
    )
    out = np.empty((B, CLS), dtype=np.float32)
    for c in range(NCORES):
        out[c * BL : (c + 1) * BL, :] = res.results[c]["out"][:CLS, :].T
    return out, res


def kernel(**inputs):
    out, _ = _run(inputs, trace=False)
    return out
